# revision 13
# baseline (speedup 1.0000x reference)
"""Trainium2 Bass kernel for 16-head causal multi-head attention.

Problem shape: B=2, S=2048, D=1024, H=16 (head dim 64), fp32.

Sharding (8 cores): core c handles batch b=c//4 and head-group hg=c%4
(4 heads = 256 features).  Host pre-transposes activations and weight
slices so every device load is contiguous; host sums the 8 partial
output projections and assembles/normalizes attention weights from the
device's unnormalized transposed exp-scores (pure gather/unshard work).

Device pipeline per core:
  1. Q^T,K^T (feat x tok) and V (tok x feat, with ones column) projections.
  2. Per (head, q-tile): transposed score tiles S^T = K^T_slice^T Q^T_slice
     on PE (causal tiles skipped), diagonal-block causal mask add (DVE),
     exp((s)*(1/8)) on ScalarE -> SBUF -> DMA to HBM (unnormalized),
     PV matmul accumulates [V|1]^T @ expT -> attn_out^T rows + denominator
     row in PSUM.
  3. Normalize attn_out^T by 1/denominator (reciprocal + gpsimd partition
     broadcast + DVE multiply); write 1/denominator to HBM for the host.
  4. Output projection partial = attn_out^T^T @ Wo[:,F]^T on PE -> HBM.

The masked upper triangle of the attention weights is never written:
run_bass_kernel_spmd zero-initializes (and donates) output buffers.
"""

import numpy as np

import concourse.bass as bass
import concourse.mybir as mybir
from concourse import tile
from concourse.bass_utils import run_bass_kernel_spmd

F32 = mybir.dt.float32
F32R = mybir.dt.float32r
AF = mybir.ActivationFunctionType


B, S, D, H = 2, 2048, 1024, 16
HD = D // H            # 64
HPC = 4                # heads per core
FPC = HPC * HD         # 256 features per core
NCORES = 8
QT = 512               # q tile width
NQT = S // QT          # 4
KB = 128               # k block height
NKB = S // KB          # 16
NDC = D // 128         # 8 contraction chunks for projections
INV_SCALE = 1.0 / float(np.sqrt(HD))


def _split_sync(nc, max_waits=1, max_updates=1):
    def _mknop(name, sync_info, engine):
        nop = mybir.InstNoOp(name=name, ins=[], outs=[])
        nop.engine = engine
        nop.sync_info = sync_info
        nc.register_instruction(nop)
        return nop

    """Split multi-wait/multi-update sync_info into single-sync NoOps.

    The walrus build in this environment accepts at most one sync wait and
    one sync update per instruction; Tile emits several.  Hoist extras onto
    adjacent same-engine NoOps (engines execute their stream in order, so
    sequential waits are equivalent to one multi-wait)."""
    n = 0
    for fn in nc.m.functions:
        for bb in fn.blocks:
            il = list(bb.instructions)
            out = []
            changed = False
            for inst in il:
                si = inst.sync_info
                ws = list(si.on_wait) if si is not None else []
                us = list(si.on_update) if si is not None else []
                pre, post = [], []
                if len(ws) > max_waits:
                    for w in ws[:-max_waits]:
                        n += 1
                        pre.append(_mknop(
                            f"syncsplitw-{n}",
                            mybir.SyncInfo(on_wait=[w], on_update=[]),
                            inst.engine,
                        ))
                    ws = ws[-max_waits:]
                    changed = True
                if len(us) > max_updates:
                    tn = type(inst).__name__
                    if "TensorLoad" in tn or "TensorSave" in tn or "Dma" in tn:
                        raise AssertionError(
                            f"multi-update on DMA inst {inst.name} ({tn})"
                        )
                    for u in us[max_updates:]:
                        n += 1
                        post.append(_mknop(
                            f"syncsplitu-{n}",
                            mybir.SyncInfo(on_wait=[], on_update=[u]),
                            inst.engine,
                        ))
                    us = us[:max_updates]
                    changed = True
                if pre or post:
                    inst.sync_info = mybir.SyncInfo(on_wait=ws, on_update=us)
                out.extend(pre)
                out.append(inst)
                out.extend(post)
            if changed:
                bb.instructions = out
    return n


class _TC(tile.TileContext):
    """TileContext whose tail drain carries one wait per instruction."""

    def _drain_and_barrier(self, tick_clock, wait_clock):
        nc = self.nc
        gc = tick_clock.global_clock
        try:
            ticks = list(gc)
        except Exception:
            ticks = eval(repr(gc).replace("VectorClock(", "").rstrip(")"))
        for proc, h in sorted(self.sems.allocated().items()):
            t = ticks[proc]
            if t > 0:
                nc.sync.wait_ge(h, t * (16 if h.name.startswith("DMA") else 1))
        nc.sync.drain()
        nc.all_engine_barrier()
        popped = nc._tile_sem_poison_stack.pop()
        assert popped is self._sem_poison
        nc.clear_and_free_semaphores(list(self.sems.allocated().values()))
        nc.all_engine_barrier()


def _build_program():
    nc = bass.Bass()

    xqT = nc.dram_tensor("xqT", [D, S], F32R, kind="ExternalInput")
    xkT = nc.dram_tensor("xkT", [D, S], F32R, kind="ExternalInput")
    xvT = nc.dram_tensor("xvT", [D, S], F32R, kind="ExternalInput")
    wqT = nc.dram_tensor("wqT", [D, FPC], F32R, kind="ExternalInput")
    wkT = nc.dram_tensor("wkT", [D, FPC], F32R, kind="ExternalInput")
    wvT = nc.dram_tensor("wvT", [D, FPC], F32R, kind="ExternalInput")
    woT = nc.dram_tensor("woT", [FPC, D], F32R, kind="ExternalInput")
    bqd = nc.dram_tensor("bqd", [2, 128], F32, kind="ExternalInput")
    bkd = nc.dram_tensor("bkd", [2, 128], F32, kind="ExternalInput")
    bvd = nc.dram_tensor("bvd", [FPC], F32, kind="ExternalInput")
    cmd = nc.dram_tensor("cmd", [128, 128], F32R, kind="ExternalInput")
    eyed = nc.dram_tensor("eyed", [128, 128], F32R, kind="ExternalInput")

    wexp = nc.dram_tensor("wexp", [HPC, S, S], F32R, kind="ExternalOutput")
    rec = nc.dram_tensor("rec", [HPC, S], F32R, kind="ExternalOutput")
    pout = nc.dram_tensor("pout", [S, D], F32, kind="ExternalOutput")

    from contextlib import ExitStack

    with _TC(nc) as tc, ExitStack() as ctx:
        pers = ctx.enter_context(tc.tile_pool(name="pers", bufs=1))
        xp = ctx.enter_context(tc.tile_pool(name="xp", bufs=9))
        ep = ctx.enter_context(tc.tile_pool(name="ep", bufs=8))
        op = ctx.enter_context(tc.tile_pool(name="op", bufs=4))
        ps = ctx.enter_context(tc.tile_pool(name="ps", bufs=6, space="PSUM"))
        psa = ctx.enter_context(tc.tile_pool(name="psa", bufs=2, space="PSUM"))

        # ---- persistent tiles ----
        wq_sb = pers.tile([128, NDC, FPC], F32R, name="wq_sb")
        wk_sb = pers.tile([128, NDC, FPC], F32R, name="wk_sb")
        wv_sb = pers.tile([128, NDC, FPC], F32R, name="wv_sb")
        wo_sb = pers.tile([128, 2, D], F32R, name="wo_sb")
        for dc in range(NDC):
            dsl = slice(128 * dc, 128 * (dc + 1))
            nc.sync.dma_start(wq_sb[:, dc, :], wqT[dsl, :])
            nc.sync.dma_start(wk_sb[:, dc, :], wkT[dsl, :])
            nc.sync.dma_start(wv_sb[:, dc, :], wvT[dsl, :])
        for dc in range(2):
            nc.sync.dma_start(wo_sb[:, dc, :], woT[128 * dc:128 * (dc + 1), :])

        bq_sb = pers.tile([128, 2], F32, name="bq_sb")
        bk_sb = pers.tile([128, 2], F32, name="bk_sb")
        nc.sync.dma_start(bq_sb, bqd[:, :].rearrange("h p -> p h"))
        nc.sync.dma_start(bk_sb, bkd[:, :].rearrange("h p -> p h"))
        bv_sb = pers.tile([128, FPC], F32, name="bv_sb")
        nc.gpsimd.dma_start(
            out=bv_sb,
            in_=bass.AP(tensor=bvd, offset=0, ap=[[0, 128], [1, FPC]]),
        )
        cm_sb = pers.tile([128, 128], F32R, name="cm_sb")
        nc.sync.dma_start(cm_sb, cmd[:, :])
        eye_sb = pers.tile([128, 128], F32R, name="eye_sb")
        nc.sync.dma_start(eye_sb, eyed[:, :])

        qT = [pers.tile([128, S], F32R, name=f"qT{hp}") for hp in range(2)]
        kT = [pers.tile([128, S], F32R, name=f"kT{hp}") for hp in range(2)]
        # V natural layout + ones columns: per hp (128 tok, 16 blk, 130):
        # cols [0:64]=head even, [64]=ones, [65:129]=head odd, [129]=ones.
        vv = [pers.tile([128, NKB, 130], F32R, name=f"vv{hp}") for hp in range(2)]
        ao = [pers.tile([128, S], F32R, name=f"ao{hp}") for hp in range(2)]
        for hp in range(2):
            nc.vector.memset(vv[hp][:, :, 64].bitcast(F32), 1.0)
            nc.vector.memset(vv[hp][:, :, 129].bitcast(F32), 1.0)
        ones_sb = pers.tile([1, 64], F32R, name="ones_sb")
        nc.vector.memset(ones_sb.bitcast(F32), 1.0)

        # ---- phase A: projections ----
        for t in range(NQT):
            tsl = slice(QT * t, QT * (t + 1))
            q_ps = [ps.tile([128, QT], F32, tag="mm", name=f"q_ps{hp}") for hp in range(2)]
            k_ps = [ps.tile([128, QT], F32, tag="mm", name=f"k_ps{hp}") for hp in range(2)]
            v_ps = [psa.tile([128, QT], F32, tag="ao", name=f"v_ps{mb}") for mb in range(2)]
            for dc in range(NDC):
                xq_td = xp.tile([128, QT], F32R, tag="xt", name="xq_td")
                xk_td = xp.tile([128, QT], F32R, tag="xt", name="xk_td")
                xv_td = xp.tile([128, QT], F32R, tag="xt", name="xv_td")
                dsl = slice(128 * dc, 128 * (dc + 1))
                nc.sync.dma_start(xq_td, xqT[dsl, tsl])
                nc.sync.dma_start(xk_td, xkT[dsl, tsl])
                nc.sync.dma_start(xv_td, xvT[dsl, tsl])
                for hp in range(2):
                    fsl = slice(128 * hp, 128 * (hp + 1))
                    nc.tensor.matmul(
                        q_ps[hp], (wq_sb[:, dc, fsl]), (xq_td),
                        start=(dc == 0), stop=(dc == NDC - 1),
                    )
                    nc.tensor.matmul(
                        k_ps[hp], (wk_sb[:, dc, fsl]), (xk_td),
                        start=(dc == 0), stop=(dc == NDC - 1),
                    )
                for m in range(4):
                    nc.tensor.matmul(
                        v_ps[m // 2][:, 256 * (m % 2):256 * (m % 2) + 256],
                        (xv_td[:, 128 * m:128 * (m + 1)]),
                        (wv_sb[:, dc, :]),
                        start=(dc == 0 and m % 2 == 0),
                        stop=(dc == NDC - 1),
                        skip_group_check=True,
                    )
            for hp in range(2):
                nc.vector.tensor_scalar_add(qT[hp][:, tsl], q_ps[hp], bq_sb[:, hp:hp + 1])
                nc.vector.tensor_scalar_add(kT[hp][:, tsl], k_ps[hp], bk_sb[:, hp:hp + 1])
            for m in range(4):
                blk = 4 * t + m
                for hp in range(2):
                    for h in range(2):
                        c0 = 256 * (m % 2) + 128 * hp + 64 * h
                        nc.vector.tensor_add(
                            vv[hp][:, blk, 65 * h:65 * h + 64],
                            v_ps[m // 2][:, c0:c0 + 64],
                            bv_sb[:, 128 * hp + 64 * h:128 * hp + 64 * h + 64],
                        )

        # ---- phases B+C: attention per q-tile, then output projection ----
        for qt in range(NQT):
            qsl_base = QT * qt
            for hp in range(2):
                # two heads of the pair run interleaved: their score matmuls
                # use disjoint PE row groups (base partition 0 vs 64) and
                # execute concurrently in the array
                ao_ps = [psa.tile([65, QT], F32, tag="ao", name=f"ao_ps{h}")
                         for h in range(2)]
                jmax = 4 * qt + 3
                for j in range(jmax + 1):
                    qoff = max(0, 128 * j - qsl_base)
                    s_ps = []
                    for h in range(2):
                        fsl = slice(64 * h, 64 * h + 64)
                        diag = j >= 4 * qt
                        sp = ps.tile([128, QT], F32, tag="mm", name=f"s_ps{h}")
                        nc.tensor.matmul(
                            sp[:, qoff:QT],
                            kT[hp][fsl, 128 * j:128 * (j + 1)],
                            qT[hp][fsl, qsl_base + qoff:qsl_base + QT],
                            start=True, stop=not diag,
                        )
                        if diag:
                            # accumulate -1e30 causal mask into the diagonal
                            # 128x128 block on the PE itself (no DVE hop)
                            nc.tensor.matmul(
                                sp[:, qoff:qoff + 128],
                                eye_sb, cm_sb,
                                start=False, stop=True,
                            )
                        s_ps.append(sp)
                    for h in range(2):
                        hd = 2 * hp + h
                        sp = s_ps[h]
                        e_sb = ep.tile([128, QT], F32R, tag="e", name="e_sb")
                        nc.scalar.activation(
                            e_sb[:, qoff:QT], sp[:, qoff:QT], AF.Exp,
                            scale=INV_SCALE,
                        )
                        nc.sync.dma_start(
                            wexp[hd, 128 * j:128 * (j + 1),
                                 qsl_base + qoff:qsl_base + QT],
                            e_sb[:, qoff:QT],
                        )
                        nc.tensor.matmul(
                            ao_ps[h][:, qoff:QT],
                            vv[hp][:, j, 65 * h:65 * h + 65],
                            e_sb[:, qoff:QT],
                            start=(j == 0), stop=(j == jmax),
                        )
                for h in range(2):
                    hd = 2 * hp + h
                    fsl = slice(64 * h, 64 * h + 64)
                    rec_sb = op.tile([1, QT], F32R, tag="rec", name="rec_sb", bufs=4)
                    lns_sb = op.tile([1, QT], F32, tag="lns", name="lns_sb", bufs=4)
                    nc.scalar.activation(lns_sb, ao_ps[h][64:65, :], AF.Ln)
                    nc.scalar.activation(rec_sb, lns_sb, AF.Exp, scale=-1.0)
                    nc.sync.dma_start(
                        rec[hd:hd + 1, qsl_base:qsl_base + QT], rec_sb
                    )
                    bc_ps = ps.tile([64, QT], F32, tag="mm", name="bc_ps")
                    nc.tensor.matmul(bc_ps, ones_sb, rec_sb, start=True, stop=True)
                    bcr = op.tile([64, QT], F32, tag="bcr", name="bcr", bufs=2)
                    nc.scalar.copy(bcr, bc_ps)
                    nc.vector.tensor_mul(
                        ao[hp][fsl, qsl_base:qsl_base + QT],
                        ao_ps[h][0:64, :],
                        bcr,
                    )
            for jt in range(2):
                for m in range(4):
                    o_ps = ps.tile([128, QT], F32, tag="mm", name="o_ps")
                    for hp in range(2):
                        nc.tensor.matmul(
                            o_ps,
                            (ao[hp][:, qsl_base + 128 * m:qsl_base + 128 * (m + 1)]),
                            (wo_sb[:, hp, 512 * jt:512 * (jt + 1)]),
                            start=(hp == 0), stop=(hp == 1),
                        )
                    o_sb = op.tile([128, QT], F32, tag="o", name="o_sb")
                    nc.scalar.copy(o_sb, o_ps)
                    nc.sync.dma_start(
                        pout[qsl_base + 128 * m:qsl_base + 128 * (m + 1),
                             512 * jt:512 * (jt + 1)],
                        o_sb,
                    )

    _split_sync(nc)
    return nc


_NC_CACHE = []


def _get_program():
    if not _NC_CACHE:
        _NC_CACHE.append(_build_program())
    return _NC_CACHE[0]


def kernel(query, key, value, mask, Wq, bq, Wk, bk, Wv, bv, Wo, bo, **kwargs):
    query = np.asarray(query, dtype=np.float32)
    key = np.asarray(key, dtype=np.float32)
    value = np.asarray(value, dtype=np.float32)
    Wq = np.asarray(Wq, dtype=np.float32)
    Wk = np.asarray(Wk, dtype=np.float32)
    Wv = np.asarray(Wv, dtype=np.float32)
    Wo = np.asarray(Wo, dtype=np.float32)
    bq = np.asarray(bq, dtype=np.float32)
    bk = np.asarray(bk, dtype=np.float32)
    bv = np.asarray(bv, dtype=np.float32)
    bo = np.asarray(bo, dtype=np.float32)

    nc = _get_program()

    # causal additive mask for a diagonal 128x128 block: row=k, col=q,
    # masked (-1e30) where k > q
    ii = np.arange(128)
    cmd = np.where(ii[:, None] > ii[None, :], -1.0e30, 0.0).astype(np.float32)
    eyed = np.eye(128, dtype=np.float32)

    xT = [np.ascontiguousarray(x.T) for x in (query, key, value)]  # per batch below
    in_maps = []
    for c in range(NCORES):
        b = c // HPC
        hg = c % HPC
        fs = slice(FPC * hg, FPC * (hg + 1))
        in_maps.append({
            "xqT": np.ascontiguousarray(query[b].T),
            "xkT": np.ascontiguousarray(key[b].T),
            "xvT": np.ascontiguousarray(value[b].T),
            "wqT": np.ascontiguousarray(Wq[fs].T),
            "wkT": np.ascontiguousarray(Wk[fs].T),
            "wvT": np.ascontiguousarray(Wv[fs].T),
            "woT": np.ascontiguousarray(Wo[:, fs].T),
            "bqd": np.ascontiguousarray(bq[fs].reshape(2, 128)),
            "bkd": np.ascontiguousarray(bk[fs].reshape(2, 128)),
            "bvd": np.ascontiguousarray(bv[fs]),
            "cmd": cmd,
            "eyed": eyed,
        })

    results = run_bass_kernel_spmd(
        nc, in_maps, core_ids=list(range(NCORES)), **kwargs
    )

    out = np.zeros((B, S, D), dtype=np.float32)
    attnw = np.empty((B, H, S, S), dtype=np.float32)
    for c in range(NCORES):
        b = c // HPC
        hg = c % HPC
        r = results.results[c]
        out[b] += r["pout"]
        wexp = r["wexp"]          # (4, S k, S q) unnormalized
        recv = r["rec"]           # (4, S)
        for hh in range(HPC):
            np.multiply(
                wexp[hh].T, recv[hh][:, None], out=attnw[b, HPC * hg + hh]
            )
    out += bo
    if kwargs:
        return (out, attnw), results
    return out, attnw


# revision 14
# speedup vs baseline: 1.1938x; 1.1938x over previous
"""Trainium2 Bass kernel for 16-head causal multi-head attention.

Problem shape: B=2, S=2048, D=1024, H=16 (head dim 64), fp32.

Sharding (8 cores): core c handles batch b=c//4 and head-group hg=c%4
(4 heads = 256 features).  Host pre-transposes activations and weight
slices so every device load is contiguous; host sums the 8 partial
output projections and assembles/normalizes attention weights from the
device's unnormalized transposed exp-scores (pure gather/unshard work).

Device pipeline per core:
  1. Q^T,K^T (feat x tok) and V (tok x feat, with ones column) projections.
  2. Per (head, q-tile): transposed score tiles S^T = K^T_slice^T Q^T_slice
     on PE (causal tiles skipped), diagonal-block causal mask add (DVE),
     exp((s)*(1/8)) on ScalarE -> SBUF -> DMA to HBM (unnormalized),
     PV matmul accumulates [V|1]^T @ expT -> attn_out^T rows + denominator
     row in PSUM.
  3. Normalize attn_out^T by 1/denominator (reciprocal + gpsimd partition
     broadcast + DVE multiply); write 1/denominator to HBM for the host.
  4. Output projection partial = attn_out^T^T @ Wo[:,F]^T on PE -> HBM.

The masked upper triangle of the attention weights is never written:
run_bass_kernel_spmd zero-initializes (and donates) output buffers.
"""

import numpy as np

import concourse.bass as bass
import concourse.mybir as mybir
from concourse import tile
from concourse.bass_utils import run_bass_kernel_spmd

F32 = mybir.dt.float32
F32R = mybir.dt.float32r
BF16 = mybir.dt.bfloat16
AF = mybir.ActivationFunctionType


B, S, D, H = 2, 2048, 1024, 16
HD = D // H            # 64
HPC = 4                # heads per core
FPC = HPC * HD         # 256 features per core
NCORES = 8
QT = 512               # q tile width
NQT = S // QT          # 4
KB = 128               # k block height
NKB = S // KB          # 16
NDC = D // 128         # 8 contraction chunks for projections
INV_SCALE = 1.0 / float(np.sqrt(HD))


def _split_sync(nc, max_waits=1, max_updates=1):
    def _mknop(name, sync_info, engine):
        nop = mybir.InstNoOp(name=name, ins=[], outs=[])
        nop.engine = engine
        nop.sync_info = sync_info
        nc.register_instruction(nop)
        return nop

    """Split multi-wait/multi-update sync_info into single-sync NoOps.

    The walrus build in this environment accepts at most one sync wait and
    one sync update per instruction; Tile emits several.  Hoist extras onto
    adjacent same-engine NoOps (engines execute their stream in order, so
    sequential waits are equivalent to one multi-wait)."""
    n = 0
    for fn in nc.m.functions:
        for bb in fn.blocks:
            il = list(bb.instructions)
            out = []
            changed = False
            for inst in il:
                si = inst.sync_info
                ws = list(si.on_wait) if si is not None else []
                us = list(si.on_update) if si is not None else []
                pre, post = [], []
                if len(ws) > max_waits:
                    for w in ws[:-max_waits]:
                        n += 1
                        pre.append(_mknop(
                            f"syncsplitw-{n}",
                            mybir.SyncInfo(on_wait=[w], on_update=[]),
                            inst.engine,
                        ))
                    ws = ws[-max_waits:]
                    changed = True
                if len(us) > max_updates:
                    tn = type(inst).__name__
                    if "TensorLoad" in tn or "TensorSave" in tn or "Dma" in tn:
                        raise AssertionError(
                            f"multi-update on DMA inst {inst.name} ({tn})"
                        )
                    for u in us[max_updates:]:
                        n += 1
                        post.append(_mknop(
                            f"syncsplitu-{n}",
                            mybir.SyncInfo(on_wait=[], on_update=[u]),
                            inst.engine,
                        ))
                    us = us[:max_updates]
                    changed = True
                if pre or post:
                    inst.sync_info = mybir.SyncInfo(on_wait=ws, on_update=us)
                out.extend(pre)
                out.append(inst)
                out.extend(post)
            if changed:
                bb.instructions = out
    return n


class _TC(tile.TileContext):
    """TileContext whose tail drain carries one wait per instruction."""

    def _drain_and_barrier(self, tick_clock, wait_clock):
        nc = self.nc
        gc = tick_clock.global_clock
        try:
            ticks = list(gc)
        except Exception:
            ticks = eval(repr(gc).replace("VectorClock(", "").rstrip(")"))
        for proc, h in sorted(self.sems.allocated().items()):
            t = ticks[proc]
            if t > 0:
                nc.sync.wait_ge(h, t * (16 if h.name.startswith("DMA") else 1))
        nc.sync.drain()
        nc.all_engine_barrier()
        popped = nc._tile_sem_poison_stack.pop()
        assert popped is self._sem_poison
        nc.clear_and_free_semaphores(list(self.sems.allocated().values()))
        nc.all_engine_barrier()


def _build_program():
    nc = bass.Bass()

    xqT = nc.dram_tensor("xqT", [D, S], F32R, kind="ExternalInput")
    xkT = nc.dram_tensor("xkT", [D, S], F32R, kind="ExternalInput")
    xvT = nc.dram_tensor("xvT", [D, S], F32R, kind="ExternalInput")
    wqT = nc.dram_tensor("wqT", [D, FPC], F32R, kind="ExternalInput")
    wkT = nc.dram_tensor("wkT", [D, FPC], F32R, kind="ExternalInput")
    wvT = nc.dram_tensor("wvT", [D, FPC], F32R, kind="ExternalInput")
    woT = nc.dram_tensor("woT", [FPC, D], F32R, kind="ExternalInput")
    bqd = nc.dram_tensor("bqd", [2, 128], F32, kind="ExternalInput")
    bkd = nc.dram_tensor("bkd", [2, 128], F32, kind="ExternalInput")
    bvd = nc.dram_tensor("bvd", [FPC], F32, kind="ExternalInput")
    cmd = nc.dram_tensor("cmd", [128, 128], F32, kind="ExternalInput")

    wexp = nc.dram_tensor("wexp", [HPC, S, S], BF16, kind="ExternalOutput")
    rec = nc.dram_tensor("rec", [HPC, S], F32R, kind="ExternalOutput")
    pout = nc.dram_tensor("pout", [S, D], F32, kind="ExternalOutput")

    from contextlib import ExitStack

    with _TC(nc) as tc, ExitStack() as ctx:
        pers = ctx.enter_context(tc.tile_pool(name="pers", bufs=1))
        xp = ctx.enter_context(tc.tile_pool(name="xp", bufs=9))
        ep = ctx.enter_context(tc.tile_pool(name="ep", bufs=12))
        op = ctx.enter_context(tc.tile_pool(name="op", bufs=4))
        ps = ctx.enter_context(tc.tile_pool(name="ps", bufs=5, space="PSUM"))
        psa = ctx.enter_context(tc.tile_pool(name="psa", bufs=3, space="PSUM"))

        # ---- persistent tiles ----
        wq_sb = pers.tile([128, NDC, FPC], F32R, name="wq_sb")
        wk_sb = pers.tile([128, NDC, FPC], F32R, name="wk_sb")
        wv_sb = pers.tile([128, NDC, FPC], F32R, name="wv_sb")
        wo_sb = pers.tile([128, 2, D], F32R, name="wo_sb")
        for dc in range(NDC):
            dsl = slice(128 * dc, 128 * (dc + 1))
            nc.sync.dma_start(wq_sb[:, dc, :], wqT[dsl, :])
            nc.sync.dma_start(wk_sb[:, dc, :], wkT[dsl, :])
            nc.sync.dma_start(wv_sb[:, dc, :], wvT[dsl, :])
        for dc in range(2):
            nc.sync.dma_start(wo_sb[:, dc, :], woT[128 * dc:128 * (dc + 1), :])

        bq_sb = pers.tile([128, 2], F32, name="bq_sb")
        bk_sb = pers.tile([128, 2], F32, name="bk_sb")
        nc.sync.dma_start(bq_sb, bqd[:, :].rearrange("h p -> p h"))
        nc.sync.dma_start(bk_sb, bkd[:, :].rearrange("h p -> p h"))
        bv_sb = pers.tile([128, FPC], F32, name="bv_sb")
        nc.gpsimd.dma_start(
            out=bv_sb,
            in_=bass.AP(tensor=bvd, offset=0, ap=[[0, 128], [1, FPC]]),
        )
        cm_sb = pers.tile([128, 128], F32, name="cm_sb")
        nc.sync.dma_start(cm_sb, cmd[:, :])

        qT = [pers.tile([128, S], BF16, name=f"qT{hp}") for hp in range(2)]
        kT = [pers.tile([128, S], BF16, name=f"kT{hp}") for hp in range(2)]
        # V natural layout + ones columns: per hp (128 tok, 16 blk, 130):
        # cols [0:64]=head even, [64]=ones, [65:129]=head odd, [129]=ones.
        vv = [pers.tile([128, NKB, 130], BF16, name=f"vv{hp}") for hp in range(2)]
        ao = [pers.tile([128, S], F32R, name=f"ao{hp}") for hp in range(2)]
        for hp in range(2):
            nc.vector.memset(vv[hp][:, :, 64], 1.0)
            nc.vector.memset(vv[hp][:, :, 129], 1.0)
        ones_sb = pers.tile([1, 64], F32R, name="ones_sb")
        nc.vector.memset(ones_sb.bitcast(F32), 1.0)

        # ---- phase A: projections ----
        for t in range(NQT):
            tsl = slice(QT * t, QT * (t + 1))
            q_ps = [ps.tile([128, QT], F32, tag="mm", name=f"q_ps{hp}") for hp in range(2)]
            k_ps = [ps.tile([128, QT], F32, tag="mm", name=f"k_ps{hp}") for hp in range(2)]
            v_ps = [psa.tile([128, QT], F32, tag="ao", name=f"v_ps{mb}") for mb in range(2)]
            for dc in range(NDC):
                xq_td = xp.tile([128, QT], F32R, tag="xt", name="xq_td")
                xk_td = xp.tile([128, QT], F32R, tag="xt", name="xk_td")
                xv_td = xp.tile([128, QT], F32R, tag="xt", name="xv_td")
                dsl = slice(128 * dc, 128 * (dc + 1))
                nc.sync.dma_start(xq_td, xqT[dsl, tsl])
                nc.sync.dma_start(xk_td, xkT[dsl, tsl])
                nc.sync.dma_start(xv_td, xvT[dsl, tsl])
                for hp in range(2):
                    fsl = slice(128 * hp, 128 * (hp + 1))
                    nc.tensor.matmul(
                        q_ps[hp], (wq_sb[:, dc, fsl]), (xq_td),
                        start=(dc == 0), stop=(dc == NDC - 1),
                    )
                    nc.tensor.matmul(
                        k_ps[hp], (wk_sb[:, dc, fsl]), (xk_td),
                        start=(dc == 0), stop=(dc == NDC - 1),
                    )
                for m in range(4):
                    nc.tensor.matmul(
                        v_ps[m // 2][:, 256 * (m % 2):256 * (m % 2) + 256],
                        (xv_td[:, 128 * m:128 * (m + 1)]),
                        (wv_sb[:, dc, :]),
                        start=(dc == 0 and m % 2 == 0),
                        stop=(dc == NDC - 1),
                        skip_group_check=True,
                    )
            for hp in range(2):
                nc.vector.tensor_scalar_add(qT[hp][:, tsl], q_ps[hp], bq_sb[:, hp:hp + 1])
                nc.vector.tensor_scalar_add(kT[hp][:, tsl], k_ps[hp], bk_sb[:, hp:hp + 1])
            for m in range(4):
                blk = 4 * t + m
                for hp in range(2):
                    for h in range(2):
                        c0 = 256 * (m % 2) + 128 * hp + 64 * h
                        nc.vector.tensor_add(
                            vv[hp][:, blk, 65 * h:65 * h + 64],
                            v_ps[m // 2][:, c0:c0 + 64],
                            bv_sb[:, 128 * hp + 64 * h:128 * hp + 64 * h + 64],
                        )

        # ---- phases B+C: attention per q-tile, then output projection ----
        for qt in range(NQT):
            qsl_base = QT * qt
            for hp in range(2):
                # two heads of the pair run interleaved: their score matmuls
                # use disjoint PE row groups (base partition 0 vs 64) and
                # execute concurrently in the array
                ao_ps = [psa.tile([65, QT], F32, tag="ao", name=f"ao_ps{h}")
                         for h in range(2)]
                jmax = 4 * qt + 3
                for j in range(jmax + 1):
                    qoff = max(0, 128 * j - qsl_base)
                    s_ps = []
                    for h in range(2):
                        fsl = slice(64 * h, 64 * h + 64)
                        sp = ps.tile([128, QT], F32, tag="mm", name=f"s_ps{h}")
                        nc.tensor.matmul(
                            sp[:, qoff:QT],
                            kT[hp][fsl, 128 * j:128 * (j + 1)],
                            qT[hp][fsl, qsl_base + qoff:qsl_base + QT],
                            start=True, stop=True,
                        )
                        s_ps.append(sp)
                    for h in range(2):
                        hd = 2 * hp + h
                        sp = s_ps[h]
                        if j >= 4 * qt:
                            nc.vector.tensor_add(
                                sp[:, qoff:qoff + 128],
                                sp[:, qoff:qoff + 128],
                                cm_sb,
                            )
                        e_sb = ep.tile([128, QT], BF16, tag="e", name="e_sb")
                        nc.scalar.activation(
                            e_sb[:, qoff:QT], sp[:, qoff:QT], AF.Exp,
                            scale=INV_SCALE,
                        )
                        nc.sync.dma_start(
                            wexp[hd, 128 * j:128 * (j + 1),
                                 qsl_base + qoff:qsl_base + QT],
                            e_sb[:, qoff:QT],
                        )
                        nc.tensor.matmul(
                            ao_ps[h][:, qoff:QT],
                            vv[hp][:, j, 65 * h:65 * h + 65],
                            e_sb[:, qoff:QT],
                            start=(j == 0), stop=(j == jmax),
                        )
                for h in range(2):
                    hd = 2 * hp + h
                    fsl = slice(64 * h, 64 * h + 64)
                    rec_sb = op.tile([1, QT], F32R, tag="rec", name="rec_sb", bufs=4)
                    lns_sb = op.tile([1, QT], F32, tag="lns", name="lns_sb", bufs=4)
                    nc.scalar.activation(lns_sb, ao_ps[h][64:65, :], AF.Ln)
                    nc.scalar.activation(rec_sb, lns_sb, AF.Exp, scale=-1.0)
                    nc.sync.dma_start(
                        rec[hd:hd + 1, qsl_base:qsl_base + QT], rec_sb
                    )
                    bc_ps = ps.tile([64, QT], F32, tag="mm", name="bc_ps")
                    nc.tensor.matmul(bc_ps, ones_sb, rec_sb, start=True, stop=True)
                    bcr = op.tile([64, QT], F32, tag="bcr", name="bcr", bufs=2)
                    nc.scalar.copy(bcr, bc_ps)
                    nc.vector.tensor_mul(
                        ao[hp][fsl, qsl_base:qsl_base + QT],
                        ao_ps[h][0:64, :],
                        bcr,
                    )
            for jt in range(2):
                for m in range(4):
                    o_ps = ps.tile([128, QT], F32, tag="mm", name="o_ps")
                    for hp in range(2):
                        nc.tensor.matmul(
                            o_ps,
                            (ao[hp][:, qsl_base + 128 * m:qsl_base + 128 * (m + 1)]),
                            (wo_sb[:, hp, 512 * jt:512 * (jt + 1)]),
                            start=(hp == 0), stop=(hp == 1),
                        )
                    o_sb = op.tile([128, QT], F32, tag="o", name="o_sb")
                    nc.scalar.copy(o_sb, o_ps)
                    nc.sync.dma_start(
                        pout[qsl_base + 128 * m:qsl_base + 128 * (m + 1),
                             512 * jt:512 * (jt + 1)],
                        o_sb,
                    )

    _split_sync(nc)
    return nc


_NC_CACHE = []


def _get_program():
    if not _NC_CACHE:
        _NC_CACHE.append(_build_program())
    return _NC_CACHE[0]


def kernel(query, key, value, mask, Wq, bq, Wk, bk, Wv, bv, Wo, bo, **kwargs):
    query = np.asarray(query, dtype=np.float32)
    key = np.asarray(key, dtype=np.float32)
    value = np.asarray(value, dtype=np.float32)
    Wq = np.asarray(Wq, dtype=np.float32)
    Wk = np.asarray(Wk, dtype=np.float32)
    Wv = np.asarray(Wv, dtype=np.float32)
    Wo = np.asarray(Wo, dtype=np.float32)
    bq = np.asarray(bq, dtype=np.float32)
    bk = np.asarray(bk, dtype=np.float32)
    bv = np.asarray(bv, dtype=np.float32)
    bo = np.asarray(bo, dtype=np.float32)

    nc = _get_program()

    # causal additive mask for a diagonal 128x128 block: row=k, col=q,
    # masked (-1e30) where k > q
    ii = np.arange(128)
    cmd = np.where(ii[:, None] > ii[None, :], -1.0e30, 0.0).astype(np.float32)

    xT = [np.ascontiguousarray(x.T) for x in (query, key, value)]  # per batch below
    in_maps = []
    for c in range(NCORES):
        b = c // HPC
        hg = c % HPC
        fs = slice(FPC * hg, FPC * (hg + 1))
        in_maps.append({
            "xqT": np.ascontiguousarray(query[b].T),
            "xkT": np.ascontiguousarray(key[b].T),
            "xvT": np.ascontiguousarray(value[b].T),
            "wqT": np.ascontiguousarray(Wq[fs].T),
            "wkT": np.ascontiguousarray(Wk[fs].T),
            "wvT": np.ascontiguousarray(Wv[fs].T),
            "woT": np.ascontiguousarray(Wo[:, fs].T),
            "bqd": np.ascontiguousarray(bq[fs].reshape(2, 128)),
            "bkd": np.ascontiguousarray(bk[fs].reshape(2, 128)),
            "bvd": np.ascontiguousarray(bv[fs]),
            "cmd": cmd,
        })

    results = run_bass_kernel_spmd(
        nc, in_maps, core_ids=list(range(NCORES)), **kwargs
    )

    out = np.zeros((B, S, D), dtype=np.float32)
    attnw = np.empty((B, H, S, S), dtype=np.float32)
    for c in range(NCORES):
        b = c // HPC
        hg = c % HPC
        r = results.results[c]
        out[b] += r["pout"]
        wexp = r["wexp"]          # (4, S k, S q) unnormalized
        recv = r["rec"]           # (4, S)
        for hh in range(HPC):
            np.multiply(
                wexp[hh].T.astype(np.float32), recv[hh][:, None],
                out=attnw[b, HPC * hg + hh],
            )
    out += bo
    if kwargs:
        return (out, attnw), results
    return out, attnw
